# revision 10
# baseline (speedup 1.0000x reference)
"""Bass/Trainium2 kernel for nn_KVPosAttentionMapping.

Reference computation (N == M so tmp = keys):
    scores   = einsum('bhnd,bhmd->bhnm', keys, keys) / sqrt(H)
    pos_term = einsum('onmp,p->onm', pos_enc, w_pos)
    aw       = scores * sum(w_pos) + pos_term + b_pos[0]    -> [B*H, N, M]
    returns (aw, q, k, v) with q/k/v plain reshapes of the inputs.

Distribution: shard the query-row axis N across the 8 cores (128 rows each,
for all 64 batch*head pairs). This minimizes traffic versus bh-sharding,
which would replicate the 192MB pos_enc read on every core.

Per-core device pipeline (all bf16 data, f32 accumulation):
    pos_term[n,m] = sum_p posw[n,p,m]       chained DVE adds (bf16 SBUF, 2x)
    per bh, per 512-bank:  PSUM <- I.T @ pos_term        (TensorE, start=True)
                           PSUM += tmpT.T @ keysT        (TensorE, start=False)
                           SBUF <- PSUM (bf16)           (ScalarE copy)
    batched DMA of 8 bh to DRAM

Host prep: fold sum(w)/sqrt(H) into the tmp operand, fold w_pos and b_pos
into pos_enc, pre-transpose keys into PE-ready layouts, cast to bf16 (final
L2 rel err ~2e-3, far inside tolerance, and halves DMA bytes).
"""

import numpy as np

B, H, N, M, DQ, DV, P = 4, 16, 1024, 1024, 64, 64, 48
BH = B * H
NCORES = 8
NS = N // NCORES  # 128 query rows per core

OUT_GROUP = 8   # bh per output DMA batch
POS_PG = 8      # p-slices per pos load tile
KEY_QG = 8      # bh-pairs per resident key tile

_CACHE = {}


def _build_bass():
    from contextlib import ExitStack

    import concourse.mybir as mybir
    import concourse.tile as tile
    from concourse import bacc
    from concourse.masks import make_identity

    bf16 = mybir.dt.bfloat16
    f32 = mybir.dt.float32

    # Bacc (not plain Bass): its compile() pass splits multi-sem waits into
    # event-semaphore instructions — raw DMA instructions allow only 1 wait.
    nc = bacc.Bacc()
    # keys2[64*(bh%2)+d, bh//2, m]: 128-partition, 64KB/partition contiguous
    keys2 = nc.declare_dram_parameter("keys2", [128, BH // 2, M], bf16, isOutput=False)
    tmpT = nc.declare_dram_parameter("tmpT", [BH, DQ, NS], bf16, isOutput=False)
    # n-major [n, p, m]: per-partition-contiguous loads, chain adds over p
    posw = nc.declare_dram_parameter("posw", [NS, P, M], bf16, isOutput=False)
    # out layout [n, bh, m] so an OUT_GROUP of bh is contiguous per partition
    awo = nc.declare_dram_parameter("awo", [NS, BH, M], bf16, isOutput=True)

    with ExitStack() as ctx:
        tc = ctx.enter_context(tile.TileContext(nc))
        const_pool = ctx.enter_context(tc.tile_pool(name="const", bufs=1))
        pos_pool = ctx.enter_context(tc.tile_pool(name="pos", bufs=2))
        pterm_pool = ctx.enter_context(tc.tile_pool(name="pterm", bufs=1))
        k_pool = ctx.enter_context(tc.tile_pool(name="keys", bufs=1))
        t_pool = ctx.enter_context(tc.tile_pool(name="tmp", bufs=1))
        psum_pool = ctx.enter_context(tc.tile_pool(name="psum", bufs=4, space="PSUM"))
        out_pool = ctx.enter_context(tc.tile_pool(name="out", bufs=3))

        ident = const_pool.tile([128, 128], bf16)
        make_identity(nc, ident[:])

        # ---- resident keys: 4 tiles x [128, 8, 1024] ----
        ktiles = []
        for kg in range(BH // 2 // KEY_QG):
            kt = k_pool.tile([128, KEY_QG, M], bf16, tag=f"kt{kg}")
            nc.sync.dma_start(
                out=kt[:], in_=keys2[:, kg * KEY_QG:(kg + 1) * KEY_QG, :]
            )
            ktiles.append(kt)

        # ---- tmpT resident: partition = (bh%2)*64 + d, free = (bh//2, n) ----
        tmpT_sb = t_pool.tile([128, BH // 2, NS], bf16)
        nc.sync.dma_start(
            out=tmpT_sb[:],
            in_=tmpT.rearrange("(q t) d n -> (t d) q n", t=2),
        )

        # ---- pos_term [128, 1024] bf16: chained adds over p ----
        pos_term = pterm_pool.tile([NS, M], bf16)
        for pg in range(P // POS_PG):
            pt = pos_pool.tile([NS, POS_PG, M], bf16, tag="pt")
            nc.sync.dma_start(
                out=pt[:], in_=posw[:, pg * POS_PG:(pg + 1) * POS_PG, :]
            )
            if pg == 0:
                nc.vector.tensor_add(out=pos_term[:], in0=pt[:, 0, :], in1=pt[:, 1, :])
                rest = range(2, POS_PG)
            else:
                rest = range(POS_PG)
            for i in rest:
                nc.vector.tensor_add(out=pos_term[:], in0=pos_term[:], in1=pt[:, i, :])

        # ---- main loop over bh ----
        for g in range(BH // OUT_GROUP):
            ot = out_pool.tile([NS, OUT_GROUP * M], bf16)
            for bi in range(OUT_GROUP):
                bh = g * OUT_GROUP + bi
                q, t = bh // 2, bh % 2
                kq = ktiles[q // KEY_QG]
                lhsT = tmpT_sb[t * DQ:(t + 1) * DQ, q, :]
                ps = psum_pool.tile([NS, M], f32)
                for j in range(M // 512):
                    js = slice(j * 512, (j + 1) * 512)
                    # preload pos_term into PSUM via identity matmul ...
                    nc.tensor.matmul(
                        out=ps[:, js], lhsT=ident[:], rhs=pos_term[:, js],
                        start=True, stop=False,
                    )
                    # ... then accumulate scores on top
                    nc.tensor.matmul(
                        out=ps[:, js],
                        lhsT=lhsT,
                        rhs=kq[t * DQ:(t + 1) * DQ, q % KEY_QG, js],
                        start=False, stop=True,
                    )
                nc.scalar.copy(out=ot[:, bi * M:(bi + 1) * M], in_=ps[:])
            nc.sync.dma_start(
                out=awo[:, g * OUT_GROUP:(g + 1) * OUT_GROUP, :],
                in_=ot.rearrange("n (b m) -> n b m", b=OUT_GROUP),
            )
    nc.finalize()
    return nc


def _prep_inputs(keys, pos_enc, w_pos, b_pos):
    """Host-side marshalling into the per-core bf16 shard arrays."""
    import ml_dtypes

    bf16 = ml_dtypes.bfloat16
    scale = float(np.sum(w_pos.astype(np.float64))) / float(np.sqrt(H))

    keys_bh = np.ascontiguousarray(
        keys.reshape(BH, M, DQ).transpose(0, 2, 1)
    )  # [BH, DQ, M] f32
    # keys2[64*(bh%2)+d, bh//2, m]
    keys2 = np.ascontiguousarray(
        keys_bh.reshape(BH // 2, 2, DQ, M).transpose(1, 2, 0, 3).reshape(128, BH // 2, M)
    ).astype(bf16)

    posw = pos_enc[0].astype(np.float32) * w_pos.astype(np.float32)[None, None, :]
    posw[..., 0] += float(b_pos[0])
    # device wants n-major [N, P, M]
    posw_t = np.ascontiguousarray(posw.transpose(0, 2, 1).astype(bf16))

    in_maps = []
    for c in range(NCORES):
        sl = slice(c * NS, (c + 1) * NS)
        tmpT_c = (keys_bh[:, :, sl] * scale).astype(bf16)
        in_maps.append({
            "keys2": keys2,
            "tmpT": np.ascontiguousarray(tmpT_c),
            "posw": np.ascontiguousarray(posw_t[sl]),
        })
    return in_maps


def run(queries, keys, values, pos_enc, w_pos, b_pos, trace=False, trace_kwargs=None):
    from concourse.bass_utils import run_bass_kernel_spmd

    if "nc" not in _CACHE:
        _CACHE["nc"] = _build_bass()
    nc = _CACHE["nc"]

    in_maps = _prep_inputs(keys, pos_enc, w_pos, b_pos)
    kw = {}
    if trace:
        kw["trace"] = True
        if trace_kwargs:
            kw.update(trace_kwargs)
    res = run_bass_kernel_spmd(nc, in_maps, list(range(NCORES)), **kw)

    # [NCORES, NS, BH, M] -> aw [BH, N, M] f32
    awo = np.stack([np.asarray(r["awo"]) for r in res.results], axis=0)
    aw = np.ascontiguousarray(
        awo.reshape(N, BH, M).transpose(1, 0, 2)
    ).astype(np.float32)

    q = queries.reshape(BH, N, DQ).astype(np.float32, copy=False)
    k = keys.reshape(BH, M, DQ).astype(np.float32, copy=False)
    v = values.reshape(BH, M, DV).astype(np.float32, copy=False)
    return (aw, q, k, v), res


def kernel(queries, keys, values, pos_enc, w_pos, b_pos, **_unused):
    queries = np.asarray(queries, dtype=np.float32)
    keys = np.asarray(keys, dtype=np.float32)
    values = np.asarray(values, dtype=np.float32)
    pos_enc = np.asarray(pos_enc, dtype=np.float32)
    w_pos = np.asarray(w_pos, dtype=np.float32)
    b_pos = np.asarray(b_pos, dtype=np.float32)
    outs, _ = run(queries, keys, values, pos_enc, w_pos, b_pos, trace=False)
    return outs


# revision 13
# speedup vs baseline: 1.5418x; 1.5418x over previous
"""Bass/Trainium2 kernel for nn_KVPosAttentionMapping.

Reference computation (N == M so tmp = keys):
    scores   = einsum('bhnd,bhmd->bhnm', keys, keys) / sqrt(H)
    pos_term = einsum('onmp,p->onm', pos_enc, w_pos)
    aw       = scores * sum(w_pos) + pos_term + b_pos[0]    -> [B*H, N, M]
    returns (aw, q, k, v) with q/k/v plain reshapes of the inputs.

Distribution: shard the query-row axis N across the 8 cores (128 rows each,
for all 64 batch*head pairs). This minimizes traffic versus bh-sharding,
which would replicate the 192MB pos_enc read on every core.

Per-core device pipeline (all bf16 data, f32 accumulation):
    pos_term[n,m] = sum_p posw[n,p,m]       chained DVE adds (bf16 SBUF, 2x)
    per bh, per 512-bank:  PSUM <- I.T @ pos_term        (TensorE, start=True)
                           PSUM += tmpT.T @ keysT        (TensorE, start=False)
                           SBUF <- PSUM (bf16)           (ScalarE copy)
    batched DMA of 8 bh to DRAM

Host prep: fold sum(w)/sqrt(H) into the tmp operand, fold w_pos and b_pos
into pos_enc, pre-transpose keys into PE-ready layouts, cast to bf16 (final
L2 rel err ~2e-3, far inside tolerance, and halves DMA bytes).
"""

import numpy as np

B, H, N, M, DQ, DV, P = 4, 16, 1024, 1024, 64, 64, 48
BH = B * H
NCORES = 8
NS = N // NCORES  # 128 query rows per core

OUT_GROUP = 8   # bh per output DMA batch
POS_PG = 8      # p-slices per pos load tile
KEY_QG = 8      # bh-pairs per resident key tile

_CACHE = {}


def _build_bass():
    from contextlib import ExitStack

    import concourse.mybir as mybir
    import concourse.tile as tile
    from concourse import bacc
    from concourse.masks import make_identity

    bf16 = mybir.dt.bfloat16
    fp16 = mybir.dt.float16
    f32 = mybir.dt.float32

    # Bacc (not plain Bass): its compile() pass splits multi-sem waits into
    # event-semaphore instructions — raw DMA instructions allow only 1 wait.
    nc = bacc.Bacc()
    # keys2[64*(bh%2)+d, bh//2, m]: 128-partition, 64KB/partition contiguous
    keys2 = nc.declare_dram_parameter("keys2", [128, BH // 2, M], bf16, isOutput=False)
    tmpT = nc.declare_dram_parameter("tmpT", [BH, DQ, NS], bf16, isOutput=False)
    # n-major [n, p, m]: per-partition-contiguous loads, chain adds over p
    posw = nc.declare_dram_parameter("posw", [NS, P, M], bf16, isOutput=False)
    # out layout [n, bh, m] so an OUT_GROUP of bh is contiguous per partition
    awo = nc.declare_dram_parameter("awo", [NS, BH, M], fp16, isOutput=True)

    with ExitStack() as ctx:
        tc = ctx.enter_context(tile.TileContext(nc))
        const_pool = ctx.enter_context(tc.tile_pool(name="const", bufs=1))
        pos_pool = ctx.enter_context(tc.tile_pool(name="pos", bufs=2))
        pterm_pool = ctx.enter_context(tc.tile_pool(name="pterm", bufs=1))
        k_pool = ctx.enter_context(tc.tile_pool(name="keys", bufs=1))
        t_pool = ctx.enter_context(tc.tile_pool(name="tmp", bufs=1))
        psum_pool = ctx.enter_context(tc.tile_pool(name="psum", bufs=4, space="PSUM"))
        out_pool = ctx.enter_context(tc.tile_pool(name="out", bufs=3))

        ident = const_pool.tile([128, 128], bf16)
        make_identity(nc, ident[:])

        # ---- resident keys: 4 tiles x [128, 8, 1024] ----
        ktiles = []
        for kg in range(BH // 2 // KEY_QG):
            kt = k_pool.tile([128, KEY_QG, M], bf16, tag=f"kt{kg}")
            nc.sync.dma_start(
                out=kt[:], in_=keys2[:, kg * KEY_QG:(kg + 1) * KEY_QG, :]
            )
            ktiles.append(kt)

        # ---- tmpT resident: partition = (bh%2)*64 + d, free = (bh//2, n) ----
        tmpT_sb = t_pool.tile([128, BH // 2, NS], bf16)
        nc.sync.dma_start(
            out=tmpT_sb[:],
            in_=tmpT.rearrange("(q t) d n -> (t d) q n", t=2),
        )

        # ---- pos_term [128, 1024] bf16: tree reduction over p (depth ~6
        #      keeps bf16 rounding ~sqrt(depth) instead of ~47 for a chain) ----
        pos_term = pterm_pool.tile([NS, M], bf16)
        partials = pterm_pool.tile([NS, P // POS_PG, M], bf16)
        NPG = P // POS_PG
        for pg in range(NPG):
            pt = pos_pool.tile([NS, POS_PG, M], bf16, tag="pt")
            nc.sync.dma_start(
                out=pt[:], in_=posw[:, pg * POS_PG:(pg + 1) * POS_PG, :]
            )
            for step in (1, 2):
                for i in range(0, POS_PG, 2 * step):
                    nc.vector.tensor_add(
                        out=pt[:, i, :], in0=pt[:, i, :], in1=pt[:, i + step, :]
                    )
            nc.vector.tensor_add(
                out=partials[:, pg, :], in0=pt[:, 0, :], in1=pt[:, 4, :]
            )
        for i in range(0, NPG, 2):
            nc.vector.tensor_add(
                out=partials[:, i, :], in0=partials[:, i, :], in1=partials[:, i + 1, :]
            )
        nc.vector.tensor_add(
            out=partials[:, 0, :], in0=partials[:, 0, :], in1=partials[:, 2, :]
        )
        nc.vector.tensor_add(
            out=pos_term[:], in0=partials[:, 0, :], in1=partials[:, 4, :]
        )

        # ---- main loop over bh: merge split DVE-add / PE-preload+ACT-copy ----
        for g in range(BH // OUT_GROUP):
            ot = out_pool.tile([NS, OUT_GROUP * M], fp16)
            for bi in range(OUT_GROUP):
                bh = g * OUT_GROUP + bi
                q, t = bh // 2, bh % 2
                kq = ktiles[q // KEY_QG]
                lhsT = tmpT_sb[t * DQ:(t + 1) * DQ, q, :]
                ps = psum_pool.tile([NS, M], f32)
                use_act = (bi % 2) == 1
                if use_act:
                    # identity loaded once for both banks, then scores
                    for j in range(M // 512):
                        js = slice(j * 512, (j + 1) * 512)
                        nc.tensor.matmul(
                            out=ps[:, js], lhsT=ident[:], rhs=pos_term[:, js],
                            start=True, stop=False,
                        )
                    for j in range(M // 512):
                        js = slice(j * 512, (j + 1) * 512)
                        nc.tensor.matmul(
                            out=ps[:, js],
                            lhsT=lhsT,
                            rhs=kq[t * DQ:(t + 1) * DQ, q % KEY_QG, js],
                            start=False, stop=True,
                        )
                    nc.scalar.copy(out=ot[:, bi * M:(bi + 1) * M], in_=ps[:])
                else:
                    for j in range(M // 512):
                        js = slice(j * 512, (j + 1) * 512)
                        nc.tensor.matmul(
                            out=ps[:, js],
                            lhsT=lhsT,
                            rhs=kq[t * DQ:(t + 1) * DQ, q % KEY_QG, js],
                            start=True, stop=True,
                        )
                    nc.vector.tensor_add(
                        out=ot[:, bi * M:(bi + 1) * M], in0=ps[:], in1=pos_term[:]
                    )
            nc.sync.dma_start(
                out=awo[:, g * OUT_GROUP:(g + 1) * OUT_GROUP, :],
                in_=ot.rearrange("n (b m) -> n b m", b=OUT_GROUP),
            )
    nc.finalize()
    return nc


def _prep_inputs(keys, pos_enc, w_pos, b_pos):
    """Host-side marshalling into the per-core bf16 shard arrays."""
    import ml_dtypes

    bf16 = ml_dtypes.bfloat16
    scale = float(np.sum(w_pos.astype(np.float64))) / float(np.sqrt(H))

    keys_bh = np.ascontiguousarray(
        keys.reshape(BH, M, DQ).transpose(0, 2, 1)
    )  # [BH, DQ, M] f32
    # keys2[64*(bh%2)+d, bh//2, m]
    keys2 = np.ascontiguousarray(
        keys_bh.reshape(BH // 2, 2, DQ, M).transpose(1, 2, 0, 3).reshape(128, BH // 2, M)
    ).astype(bf16)

    posw = pos_enc[0].astype(np.float32) * w_pos.astype(np.float32)[None, None, :]
    posw[..., 0] += float(b_pos[0])
    # device wants n-major [N, P, M]
    posw_t = np.ascontiguousarray(posw.transpose(0, 2, 1).astype(bf16))

    in_maps = []
    for c in range(NCORES):
        sl = slice(c * NS, (c + 1) * NS)
        tmpT_c = (keys_bh[:, :, sl] * scale).astype(bf16)
        in_maps.append({
            "keys2": keys2,
            "tmpT": np.ascontiguousarray(tmpT_c),
            "posw": np.ascontiguousarray(posw_t[sl]),
        })
    return in_maps


def run(queries, keys, values, pos_enc, w_pos, b_pos, trace=False, trace_kwargs=None):
    from concourse.bass_utils import run_bass_kernel_spmd

    if "nc" not in _CACHE:
        _CACHE["nc"] = _build_bass()
    nc = _CACHE["nc"]

    in_maps = _prep_inputs(keys, pos_enc, w_pos, b_pos)
    kw = {}
    if trace:
        kw["trace"] = True
        if trace_kwargs:
            kw.update(trace_kwargs)
    res = run_bass_kernel_spmd(nc, in_maps, list(range(NCORES)), **kw)

    # [NCORES, NS, BH, M] -> aw [BH, N, M] f32
    awo = np.stack([np.asarray(r["awo"]) for r in res.results], axis=0)
    aw = np.ascontiguousarray(
        awo.reshape(N, BH, M).transpose(1, 0, 2)
    ).astype(np.float32)

    q = queries.reshape(BH, N, DQ).astype(np.float32, copy=False)
    k = keys.reshape(BH, M, DQ).astype(np.float32, copy=False)
    v = values.reshape(BH, M, DV).astype(np.float32, copy=False)
    return (aw, q, k, v), res


def kernel(queries, keys, values, pos_enc, w_pos, b_pos, **_unused):
    queries = np.asarray(queries, dtype=np.float32)
    keys = np.asarray(keys, dtype=np.float32)
    values = np.asarray(values, dtype=np.float32)
    pos_enc = np.asarray(pos_enc, dtype=np.float32)
    w_pos = np.asarray(w_pos, dtype=np.float32)
    b_pos = np.asarray(b_pos, dtype=np.float32)
    outs, _ = run(queries, keys, values, pos_enc, w_pos, b_pos, trace=False)
    return outs
